# revision 9
# baseline (speedup 1.0000x reference)
"""Multi-head attention (B=2, S=2048, H=16, D=64) on 8 Trainium2 NeuronCores.

Head-parallel tensor parallelism: core c owns heads {2c, 2c+1} (a 128-dim
slice of the model dim): column-parallel QKV projections and local causal
attention for its 2 heads, AllToAll of bf16 context vectors (one 512-token
chunk at a time), then each core runs the full-width Wo projection for its
own disjoint 64-token slices.

Key structural choices (v2, rebuilt from a traced v1):
- DMA order: packed qkv weights + biases + tri first (~0.8 MB), then x^T in
  1024-token chunks (2 KB contiguous rows per partition line), then Wo.
  The first projection matmul can start ~7 us in, instead of waiting ~50 us
  behind an 8 MB x load with 1 KB lines.
- Scores for the two heads run CONCURRENTLY as row-tiled K=64 matmuls
  (tile_position (0,0)/(64,0)) into the two banks of one [128,1024] PSUM
  tile; one paired Exp activation covers both banks.
- Attention-times-V uses V as the stationary operand ([keys, 64+ones]) so
  the PE streams exp columns and emits ctx^T [dims, queries] directly into
  a PSUM accumulator over key blocks -- no per-block LDWEIGHTS bound and no
  128x128 context transposes afterwards. The ones column gives the softmax
  denominator on PSUM partition 64.
- Normalization multiplies ctx^T by a DMA-broadcast reciprocal row
  (reciprocal_approx_fast), pipelined one query-group behind attention.
- V is transposed into [keys, dims] tiles by the DMA transpose XBAR
  (SBUF->SBUF), costing no PE/DVE time.
- Emission interleaves batch-0 attention with batch-1 QKV tiles and batch-1
  attention with batch-0 output projections so the scalar-engine exp stream
  hides under projection matmuls; per-512-token AllToAlls keep the final
  exchange small.
"""

import sys

sys.path.insert(0, "/opt/trn_rl_repo")

import ml_dtypes
import numpy as np

import concourse.bass as bass
import concourse.tile as tile
from concourse import bacc, mybir
from concourse.bass_utils import run_bass_kernel_spmd

N_CORES = 8
B, S, H, D = 2, 2048, 16, 64
E = H * D            # 1024
T = B * S            # 4096 tokens
DPC = 128            # dims (2 heads) per core
NKC = E // 128       # 8 contraction chunks for the projections
NTT = T // 512       # 8 token tiles of 512
NTB = T // 128       # 32 token blocks of 128
SB = S // 128        # 16 key blocks per batch

F32 = mybir.dt.float32
BF16 = mybir.dt.bfloat16
AFT = mybir.ActivationFunctionType


def build_program():
    nc = bacc.Bacc("TRN2", target_bir_lowering=False, debug=False,
                   num_devices=N_CORES)

    xT = nc.dram_tensor("xT", [E, T], BF16, kind="ExternalInput").ap()
    w3 = nc.dram_tensor("w3", [E, 3 * DPC], BF16, kind="ExternalInput").ap()
    woT = nc.dram_tensor("woT", [E, E], BF16, kind="ExternalInput").ap()
    b3 = nc.dram_tensor("b3", [DPC, 3], F32, kind="ExternalInput").ap()
    bo = nc.dram_tensor("bo", [E], F32, kind="ExternalInput").ap()
    # 128x128 lower-triangular (k_local <= q_local) mask
    tri = nc.dram_tensor("tri", [128, 128], BF16, kind="ExternalInput").ap()
    out = nc.dram_tensor("out", [T // N_CORES, E], F32, kind="ExternalOutput").ap()

    with tile.TileContext(nc) as tc:
        with (
            tc.tile_pool(name="consts", bufs=1) as consts,
            tc.tile_pool(name="state", bufs=1) as state,
            tc.tile_pool(name="ep", bufs=3) as ep,
            tc.tile_pool(name="rp", bufs=2) as rp,
            tc.tile_pool(name="cgp", bufs=2) as cgp,
            tc.tile_pool(name="op", bufs=2) as op,
            tc.tile_pool(name="ps_s", bufs=2, space="PSUM") as ps_s,
            tc.tile_pool(name="ps_c", bufs=4, space="PSUM") as ps_c,
            tc.tile_pool(name="dram", bufs=1, space="DRAM") as dram,
        ):
            # ---- small constants first: nothing blocks on the big x load --
            w3_sb = consts.tile([128, NKC, 3 * DPC], BF16)
            nc.sync.dma_start(
                out=w3_sb[:],
                in_=bass.AP(tensor=w3.tensor, offset=w3.offset,
                            ap=[[3 * DPC, 128], [3 * DPC * 128, NKC],
                                [1, 3 * DPC]]),
            )
            b3_sb = consts.tile([128, 3], F32)
            nc.sync.dma_start(out=b3_sb[:], in_=b3[:])
            tri_sb = consts.tile([128, 128], BF16)
            nc.sync.dma_start(out=tri_sb[:], in_=tri[:])

            # exp table warm-up (ACT set load ~2.7us) off the critical path
            warm = consts.tile([128, 1], BF16)
            nc.scalar.activation(warm[:], b3_sb[:, 0:1], AFT.Exp)

            # ---- x^T in 1024-token chunks: 2KB contiguous lines ----------
            x_sb = state.tile([128, NKC, T], BF16)  # full x^T in SBUF
            for ch in range(4):
                cs = slice(ch * 1024, (ch + 1) * 1024)
                for kc in range(NKC):
                    nc.sync.dma_start(
                        out=x_sb[:, kc, cs],
                        in_=xT[kc * 128:(kc + 1) * 128, cs])

            # ---- wo + bo after x (needed only at projection time) --------
            wo_sb = consts.tile([128, NKC, E], BF16)
            for kc in range(NKC):
                nc.sync.dma_start(out=wo_sb[:, kc, :],
                                  in_=woT[kc * 128:(kc + 1) * 128, :])
            bo_bc = consts.tile([128, E], F32)
            nc.sync.dma_start(
                out=bo_bc[:],
                in_=bass.AP(tensor=bo.tensor, offset=bo.offset,
                            ap=[[0, 128], [1, E]]),
            )

            # ---- persistent activations ----------------------------------
            qT_sb = state.tile([128, T], BF16)   # [2-head dims, tokens]
            kT_sb = state.tile([128, T], BF16)
            # [keys, 132]: [v_h0 64 | ones | pad | v_h1 64 | ones | pad]
            vN_sb = state.tile([128, NTB, 132], BF16)
            ctxT_sb = state.tile([128, T], BF16)  # normalized ctx^T

            nc.vector.memset(vN_sb[:, :, 64:65], 1.0)
            nc.vector.memset(vN_sb[:, :, 130:131], 1.0)

            # bv broadcast across partitions (v-direct output is [tok, dim])
            bv_bc = consts.tile([128, DPC], F32)
            nc.sync.dma_start(
                out=bv_bc[:],
                in_=bass.AP(tensor=b3.tensor, offset=b3.offset + 2,
                            ap=[[0, 128], [3, DPC]]),
            )

            # ---- stage A: QKV projections --------------------------------
            def emit_qkv(tt):
                ts = slice(tt * 512, (tt + 1) * 512)
                for wi, dst in ((0, qT_sb), (1, kT_sb)):
                    ps = ps_c.tile([128, 512], F32, tag="ctx", name="qkv_ps")
                    for kc in range(NKC):
                        nc.tensor.matmul(
                            ps[:], w3_sb[:, kc, wi * 128:(wi + 1) * 128],
                            x_sb[:, kc, ts],
                            start=(kc == 0), stop=(kc == NKC - 1))
                    nc.vector.tensor_scalar_add(dst[:, ts], ps[:],
                                                b3_sb[:, wi:wi + 1])
                # v directly transposed: x-block stationary -> [tokens, dims]
                for tb in range(tt * 4, tt * 4 + 4):
                    pv = ps_c.tile([128, 512], F32, tag="ctx", name="v_ps")
                    for kc in range(NKC):
                        nc.tensor.matmul(
                            pv[:, 0:DPC],
                            x_sb[:, kc, tb * 128:(tb + 1) * 128],
                            w3_sb[:, kc, 2 * 128:3 * 128],
                            start=(kc == 0), stop=(kc == NKC - 1))
                    for h in range(2):
                        nc.vector.tensor_add(
                            vN_sb[:, tb, h * 66:h * 66 + 64],
                            pv[:, h * 64:(h + 1) * 64],
                            bv_bc[:, h * 64:(h + 1) * 64])

            # ---- attention for one 512-query group -----------------------
            a2a_recv = {}
            pending = []  # (tag, fin-closure)

            def emit_attn(b, qt):
                t0 = b * S
                q0 = t0 + qt * 512
                nkb = 4 * qt + 4
                cn = [ps_c.tile([128, 512], F32, tag="ctx", name=f"cn{h}")
                      for h in range(2)]

                def emit_score_pair(kb):
                    c0 = max(kb - 4 * qt, 0) * 128
                    sp = ps_s.tile([128, 2, 512], F32, tag="sc", name="sp")
                    for h in range(2):
                        nc.tensor.matmul(
                            sp[:, h, c0:512],
                            kT_sb[h * 64:(h + 1) * 64,
                                  t0 + kb * 128:t0 + (kb + 1) * 128],
                            qT_sb[h * 64:(h + 1) * 64, q0 + c0:q0 + 512],
                            start=True, stop=True)
                    return sp

                s_tiles = {0: emit_score_pair(0)}
                for kb in range(nkb):
                    c0 = max(kb - 4 * qt, 0) * 128
                    if kb + 1 < nkb:
                        s_tiles[kb + 1] = emit_score_pair(kb + 1)
                    sp = s_tiles.pop(kb)
                    e2 = ep.tile([128, 2, 512], BF16, tag="e2", name="e2")
                    # one Exp over both heads' banks
                    nc.scalar.activation(e2[:, :, c0:512], sp[:, :, c0:512],
                                         AFT.Exp, scale=0.125)
                    if kb >= 4 * qt:  # triangular block on the diagonal
                        for h in range(2):
                            nc.vector.tensor_mul(
                                e2[:, h, c0:c0 + 128],
                                e2[:, h, c0:c0 + 128], tri_sb[:])
                    tb = b * SB + kb
                    for h in range(2):
                        nc.tensor.matmul(
                            cn[h][0:65, c0:512],
                            vN_sb[:, tb, h * 66:h * 66 + 65],
                            e2[:, h, c0:512],
                            start=(kb == 0), stop=(kb == nkb - 1),
                            skip_group_check=True)

                # reciprocals + DRAM-bounce broadcast; the ctx multiply is
                # deferred one group (fin) so the broadcast DMA is hidden
                dn2 = rp.tile([65, 512], F32, tag="dn2", name="dn2")
                rec2 = rp.tile([65, 512], F32, tag="rec2", name="rec2")
                for h in range(2):
                    nc.vector.tensor_copy(dn2[h * 64:h * 64 + 1, :],
                                          cn[h][64:65, 0:512])
                    nc.vector.reciprocal(rec2[h * 64:h * 64 + 1, :],
                                         dn2[h * 64:h * 64 + 1, :])
                recd = dram.tile([2, 512], F32, tag="recd", name="recd",
                                 bufs=4)
                for h in range(2):
                    nc.sync.dma_start(out=recd[h:h + 1, :],
                                      in_=rec2[h * 64:h * 64 + 1, :])
                rbc = rp.tile([128, 512], F32, tag="rbc", name="rbc")
                for h in range(2):
                    src = recd[h:h + 1, :]
                    nc.sync.dma_start(
                        out=rbc[h * 64:(h + 1) * 64, :],
                        in_=bass.AP(tensor=src.tensor, offset=src.offset,
                                    ap=[[0, 64], [1, 512]]))

                def fin():
                    for h in range(2):
                        nc.vector.tensor_mul(
                            ctxT_sb[h * 64:(h + 1) * 64, q0:q0 + 512],
                            cn[h][0:64, 0:512],
                            rbc[h * 64:(h + 1) * 64, :])
                    ctxd = dram.tile([N_CORES, 128, 64], BF16, tag="ctxd",
                                     name="ctxd", bufs=4)
                    for j in range(N_CORES):
                        nc.sync.dma_start(
                            out=ctxd[j],
                            in_=ctxT_sb[:, q0 + j * 64:q0 + (j + 1) * 64])
                    recv = dram.tile([N_CORES, 128, 64], BF16, tag="recv",
                                     name="recv", bufs=4)
                    nc.gpsimd.collective_compute(
                        "AllToAll",
                        mybir.AluOpType.bypass,
                        replica_groups=[list(range(N_CORES))],
                        ins=[ctxd.opt()],
                        outs=[recv.opt()],
                    )
                    a2a_recv[(b, qt)] = recv

                pending.append((("attn", b, qt), fin))

            # ---- full-width output projection for a 128-token slice ------
            def emit_proj(b, qp):
                cg = cgp.tile([128, NKC, 128], BF16, tag="cg", name="cg")
                for s2 in range(2):
                    recv = a2a_recv.pop((b, 2 * qp + s2))
                    for j in range(N_CORES):
                        nc.sync.dma_start(
                            out=cg[:, j, s2 * 64:(s2 + 1) * 64],
                            in_=recv[j])
                o_sb = op.tile([128, E], F32, tag="o", name="o_sb")
                for et in range(2):
                    ps = ps_s.tile([128, 2, 512], F32, tag="sc", name="pj_ps")
                    for kc in range(NKC):
                        nc.tensor.matmul(
                            ps[:, 0, :],
                            cg[:, kc, :],
                            wo_sb[:, kc, et * 512:(et + 1) * 512],
                            start=(kc == 0), stop=(kc == NKC - 1))
                    nc.vector.tensor_add(
                        o_sb[:, et * 512:(et + 1) * 512], ps[:, 0, :],
                        bo_bc[:, et * 512:(et + 1) * 512])
                r0 = (b * 2 + qp) * 128
                nc.sync.dma_start(out=out[r0:r0 + 128, :], in_=o_sb[:])

            # ---- emission schedule ---------------------------------------
            steps = [
                ("qkv", 0), ("qkv", 1), ("qkv", 2), ("qkv", 3),
                ("attn", 0, 0), ("qkv", 4), ("attn", 0, 1), ("qkv", 5),
                ("attn", 0, 2), ("qkv", 6), ("attn", 0, 3), ("qkv", 7),
                ("attn", 1, 0), ("proj", 0, 0), ("attn", 1, 1),
                ("proj", 0, 1), ("attn", 1, 2), ("attn", 1, 3),
                ("proj", 1, 0), ("proj", 1, 1),
            ]
            for st in steps:
                if st[0] == "qkv":
                    emit_qkv(st[1])
                elif st[0] == "attn":
                    emit_attn(st[1], st[2])
                else:
                    emit_proj(st[1], st[2])
                # flush the previous attention group's deferred finish
                while pending and pending[0][0] != st:
                    pending.pop(0)[1]()
            while pending:
                pending.pop(0)[1]()

    nc.compile()
    return nc


_NC = None


def _get_program():
    global _NC
    if _NC is None:
        _NC = build_program()
    return _NC


def _bf(a):
    return np.ascontiguousarray(a).astype(ml_dtypes.bfloat16)


def kernel(x, Wq, bq, Wk, bk, Wv, bv, Wo, bo, _trace=False, _trace_kwargs=None):
    x = np.asarray(x, np.float32)
    Wq, Wk, Wv, Wo = (np.asarray(w, np.float32) for w in (Wq, Wk, Wv, Wo))
    bq, bk, bv, bo = (np.asarray(v, np.float32) for v in (bq, bk, bv, bo))

    xT = _bf(x.reshape(T, E).T)
    i = np.arange(128)
    tri = _bf((i[:, None] <= i[None, :]).astype(np.float32))

    in_maps = []
    for c in range(N_CORES):
        sl = slice(c * DPC, (c + 1) * DPC)
        w3 = np.concatenate([Wq[sl, :].T, Wk[sl, :].T, Wv[sl, :].T], axis=1)
        b3 = np.stack([bq[sl], bk[sl], bv[sl]], axis=1)
        in_maps.append({
            "xT": xT,
            "w3": _bf(w3),
            "woT": _bf(Wo.T),
            "b3": np.ascontiguousarray(b3, np.float32),
            "bo": bo,
            "tri": tri,
        })

    nc = _get_program()
    res = run_bass_kernel_spmd(nc, in_maps, list(range(N_CORES)),
                               trace=_trace, **(_trace_kwargs or {}))
    # out[c] rows are [b, qp, s, 64]: row (b, qp, s, r) holds global token
    # b*2048 + (2*qp+s)*512 + c*64 + r.
    stacked = np.stack([res.results[i]["out"].reshape(B, 2, 2, 64, E)
                        for i in range(N_CORES)], axis=3)
    full = stacked.reshape(T, E)
    if _trace:
        return full.reshape(B, S, E), res
    return full.reshape(B, S, E)


# revision 12
# speedup vs baseline: 1.0572x; 1.0572x over previous
"""Multi-head attention (B=2, S=2048, H=16, D=64) on 8 Trainium2 NeuronCores.

Head-parallel tensor parallelism: core c owns heads {2c, 2c+1} (a 128-dim
slice of the model dim): column-parallel QKV projections and local causal
attention for its 2 heads, AllToAll of bf16 context vectors (one 512-token
chunk at a time), then each core runs the full-width Wo projection for its
own disjoint 64-token slices.

Key structural choices (v2, rebuilt from a traced v1):
- DMA order: packed qkv weights + biases + tri first (~0.8 MB), then x^T in
  1024-token chunks (2 KB contiguous rows per partition line), then Wo.
  The first projection matmul can start ~7 us in, instead of waiting ~50 us
  behind an 8 MB x load with 1 KB lines.
- Scores for the two heads run CONCURRENTLY as row-tiled K=64 matmuls
  (tile_position (0,0)/(64,0)) into the two banks of one [128,1024] PSUM
  tile; one paired Exp activation covers both banks.
- Attention-times-V uses V as the stationary operand ([keys, 64+ones]) so
  the PE streams exp columns and emits ctx^T [dims, queries] directly into
  a PSUM accumulator over key blocks -- no per-block LDWEIGHTS bound and no
  128x128 context transposes afterwards. The ones column gives the softmax
  denominator on PSUM partition 64.
- Normalization multiplies ctx^T by a DMA-broadcast reciprocal row
  (reciprocal_approx_fast), pipelined one query-group behind attention.
- V is transposed into [keys, dims] tiles by the DMA transpose XBAR
  (SBUF->SBUF), costing no PE/DVE time.
- Emission interleaves batch-0 attention with batch-1 QKV tiles and batch-1
  attention with batch-0 output projections so the scalar-engine exp stream
  hides under projection matmuls; per-512-token AllToAlls keep the final
  exchange small.
"""

import sys

sys.path.insert(0, "/opt/trn_rl_repo")

import ml_dtypes
import numpy as np

import concourse.bass as bass
import concourse.tile as tile
from concourse import bacc, mybir
from concourse.bass_utils import run_bass_kernel_spmd

N_CORES = 8
B, S, H, D = 2, 2048, 16, 64
E = H * D            # 1024
T = B * S            # 4096 tokens
DPC = 128            # dims (2 heads) per core
NKC = E // 128       # 8 contraction chunks for the projections
NTT = T // 512       # 8 token tiles of 512
NTB = T // 128       # 32 token blocks of 128
SB = S // 128        # 16 key blocks per batch

F32 = mybir.dt.float32
BF16 = mybir.dt.bfloat16
AFT = mybir.ActivationFunctionType


def build_program():
    nc = bacc.Bacc("TRN2", target_bir_lowering=False, debug=False,
                   num_devices=N_CORES)

    xT = nc.dram_tensor("xT", [E, T], BF16, kind="ExternalInput").ap()
    w3 = nc.dram_tensor("w3", [E, 3 * DPC], BF16, kind="ExternalInput").ap()
    woT = nc.dram_tensor("woT", [E, E], BF16, kind="ExternalInput").ap()
    b3 = nc.dram_tensor("b3", [DPC, 3], F32, kind="ExternalInput").ap()
    bvr = nc.dram_tensor("bvr", [DPC], F32, kind="ExternalInput").ap()
    bo = nc.dram_tensor("bo", [E], F32, kind="ExternalInput").ap()
    # 128x128 lower-triangular (k_local <= q_local) mask
    tri = nc.dram_tensor("tri", [128, 128], BF16, kind="ExternalInput").ap()
    out = nc.dram_tensor("out", [T // N_CORES, E], F32, kind="ExternalOutput").ap()

    with tile.TileContext(nc) as tc:
        with (
            tc.tile_pool(name="consts", bufs=1) as consts,
            tc.tile_pool(name="state", bufs=1) as state,
            tc.tile_pool(name="ep", bufs=3) as ep,
            tc.tile_pool(name="rp", bufs=2) as rp,
            tc.tile_pool(name="cgp", bufs=2) as cgp,
            tc.tile_pool(name="op", bufs=2) as op,
            tc.tile_pool(name="ps_s", bufs=2, space="PSUM") as ps_s,
            tc.tile_pool(name="ps_c", bufs=4, space="PSUM") as ps_c,
            tc.tile_pool(name="dram", bufs=1, space="DRAM") as dram,
        ):
            # ---- small constants first: nothing blocks on the big x load --
            w3_sb = consts.tile([128, NKC, 3 * DPC], BF16)
            nc.sync.dma_start(
                out=w3_sb[:],
                in_=bass.AP(tensor=w3.tensor, offset=w3.offset,
                            ap=[[3 * DPC, 128], [3 * DPC * 128, NKC],
                                [1, 3 * DPC]]),
            )
            b3_sb = consts.tile([128, 3], F32)
            nc.sync.dma_start(out=b3_sb[:], in_=b3[:])
            tri_sb = consts.tile([128, 128], BF16)
            nc.sync.dma_start(out=tri_sb[:], in_=tri[:])

            # exp table warm-up (ACT set load ~2.7us) off the critical path
            warm = consts.tile([128, 1], BF16)
            nc.scalar.activation(warm[:], b3_sb[:, 0:1], AFT.Exp)

            # ---- x^T in 1024-token chunks: 2KB contiguous lines ----------
            x_sb = state.tile([128, NKC, T], BF16)  # full x^T in SBUF
            for ch in range(4):
                cs = slice(ch * 1024, (ch + 1) * 1024)
                for kc in range(NKC):
                    nc.sync.dma_start(
                        out=x_sb[:, kc, cs],
                        in_=xT[kc * 128:(kc + 1) * 128, cs])

            # ---- wo + bo after x (needed only at projection time) --------
            wo_sb = consts.tile([128, NKC, E], BF16)
            for kc in range(NKC):
                nc.sync.dma_start(out=wo_sb[:, kc, :],
                                  in_=woT[kc * 128:(kc + 1) * 128, :])
            bo_bc = consts.tile([128, E], F32)
            nc.sync.dma_start(
                out=bo_bc[:],
                in_=bass.AP(tensor=bo.tensor, offset=bo.offset,
                            ap=[[0, 128], [1, E]]),
            )

            # ---- persistent activations ----------------------------------
            qT_sb = state.tile([128, T], BF16)   # [2-head dims, tokens]
            kT_sb = state.tile([128, T], BF16)
            # [keys, 132]: [v_h0 64 | ones | pad | v_h1 64 | ones | pad]
            vN_sb = state.tile([128, NTB, 132], BF16)
            ctxT_sb = state.tile([128, T], BF16)  # normalized ctx^T

            nc.vector.memset(vN_sb[:, :, 64:65], 1.0)
            nc.vector.memset(vN_sb[:, :, 130:131], 1.0)

            # bv broadcast across partitions (v-direct output is [tok, dim])
            bv_bc = consts.tile([128, DPC], F32)
            nc.sync.dma_start(
                out=bv_bc[:],
                in_=bass.AP(tensor=bvr.tensor, offset=bvr.offset,
                            ap=[[0, 128], [1, DPC]]),
            )

            # ---- stage A: QKV projections --------------------------------
            def emit_qkv(tt):
                ts = slice(tt * 512, (tt + 1) * 512)
                for wi, dst in ((0, qT_sb), (1, kT_sb)):
                    ps = ps_c.tile([128, 512], F32, tag="ctx", name="qkv_ps")
                    for kc in range(NKC):
                        nc.tensor.matmul(
                            ps[:], w3_sb[:, kc, wi * 128:(wi + 1) * 128],
                            x_sb[:, kc, ts],
                            start=(kc == 0), stop=(kc == NKC - 1))
                    nc.vector.tensor_scalar_add(dst[:, ts], ps[:],
                                                b3_sb[:, wi:wi + 1])
                # v directly transposed: x-block stationary -> [tokens, dims]
                for tb in range(tt * 4, tt * 4 + 4):
                    pv = ps_c.tile([128, 512], F32, tag="ctx", name="v_ps")
                    for kc in range(NKC):
                        nc.tensor.matmul(
                            pv[:, 0:DPC],
                            x_sb[:, kc, tb * 128:(tb + 1) * 128],
                            w3_sb[:, kc, 2 * 128:3 * 128],
                            start=(kc == 0), stop=(kc == NKC - 1))
                    for h in range(2):
                        nc.vector.tensor_add(
                            vN_sb[:, tb, h * 66:h * 66 + 64],
                            pv[:, h * 64:(h + 1) * 64],
                            bv_bc[:, h * 64:(h + 1) * 64])

            # ---- attention for one 512-query group -----------------------
            a2a_recv = {}
            pending = []  # (tag, fin-closure)

            def emit_attn(b, qt):
                t0 = b * S
                q0 = t0 + qt * 512
                nkb = 4 * qt + 4
                cn = [ps_c.tile([128, 512], F32, tag="ctx", name=f"cn{h}")
                      for h in range(2)]

                def emit_score_pair(kb):
                    c0 = max(kb - 4 * qt, 0) * 128
                    sp = ps_s.tile([128, 2, 512], F32, tag="sc", name="sp")
                    for h in range(2):
                        nc.tensor.matmul(
                            sp[:, h, c0:512],
                            kT_sb[h * 64:(h + 1) * 64,
                                  t0 + kb * 128:t0 + (kb + 1) * 128],
                            qT_sb[h * 64:(h + 1) * 64, q0 + c0:q0 + 512],
                            start=True, stop=True)
                    return sp

                s_tiles = {0: emit_score_pair(0)}
                for kb in range(nkb):
                    c0 = max(kb - 4 * qt, 0) * 128
                    if kb + 1 < nkb:
                        s_tiles[kb + 1] = emit_score_pair(kb + 1)
                    sp = s_tiles.pop(kb)
                    e2 = ep.tile([128, 2, 512], BF16, tag="e2", name="e2")
                    # one Exp over both heads' banks
                    nc.scalar.activation(e2[:, :, c0:512], sp[:, :, c0:512],
                                         AFT.Exp, scale=0.125)
                    if kb >= 4 * qt:  # triangular block on the diagonal
                        for h in range(2):
                            nc.vector.tensor_mul(
                                e2[:, h, c0:c0 + 128],
                                e2[:, h, c0:c0 + 128], tri_sb[:])
                    tb = b * SB + kb
                    for h in range(2):
                        nc.tensor.matmul(
                            cn[h][0:65, c0:512],
                            vN_sb[:, tb, h * 66:h * 66 + 65],
                            e2[:, h, c0:512],
                            start=(kb == 0), stop=(kb == nkb - 1),
                            skip_group_check=True)

                # reciprocals + DRAM-bounce broadcast; the ctx multiply is
                # deferred one group (fin) so the broadcast DMA is hidden
                dn2 = rp.tile([65, 512], F32, tag="dn2", name="dn2")
                rec2 = rp.tile([65, 512], F32, tag="rec2", name="rec2")
                for h in range(2):
                    nc.vector.tensor_copy(dn2[h * 64:h * 64 + 1, :],
                                          cn[h][64:65, 0:512])
                    nc.vector.reciprocal(rec2[h * 64:h * 64 + 1, :],
                                         dn2[h * 64:h * 64 + 1, :])
                recd = dram.tile([2, 512], F32, tag="recd", name="recd",
                                 bufs=4)
                for h in range(2):
                    nc.sync.dma_start(out=recd[h:h + 1, :],
                                      in_=rec2[h * 64:h * 64 + 1, :])
                rbc = rp.tile([128, 512], F32, tag="rbc", name="rbc")
                for h in range(2):
                    src = recd[h:h + 1, :]
                    nc.sync.dma_start(
                        out=rbc[h * 64:(h + 1) * 64, :],
                        in_=bass.AP(tensor=src.tensor, offset=src.offset,
                                    ap=[[0, 64], [1, 512]]))

                def fin():
                    for h in range(2):
                        nc.vector.tensor_mul(
                            ctxT_sb[h * 64:(h + 1) * 64, q0:q0 + 512],
                            cn[h][0:64, 0:512],
                            rbc[h * 64:(h + 1) * 64, :])
                    ctxd = dram.tile([N_CORES, 128, 64], BF16, tag="ctxd",
                                     name="ctxd", bufs=4)
                    for j in range(N_CORES):
                        nc.sync.dma_start(
                            out=ctxd[j],
                            in_=ctxT_sb[:, q0 + j * 64:q0 + (j + 1) * 64])
                    recv = dram.tile([N_CORES, 128, 64], BF16, tag="recv",
                                     name="recv", bufs=4)
                    nc.gpsimd.collective_compute(
                        "AllToAll",
                        mybir.AluOpType.bypass,
                        replica_groups=[list(range(N_CORES))],
                        ins=[ctxd.opt()],
                        outs=[recv.opt()],
                    )
                    a2a_recv[(b, qt)] = recv

                pending.append((("attn", b, qt), fin))

            # ---- full-width output projection for a 128-token slice ------
            def emit_proj(b, qp):
                cg = cgp.tile([128, NKC, 128], BF16, tag="cg", name="cg")
                for s2 in range(2):
                    recv = a2a_recv.pop((b, 2 * qp + s2))
                    for j in range(N_CORES):
                        nc.sync.dma_start(
                            out=cg[:, j, s2 * 64:(s2 + 1) * 64],
                            in_=recv[j])
                o_sb = op.tile([128, E], F32, tag="o", name="o_sb")
                for et in range(2):
                    ps = ps_s.tile([128, 2, 512], F32, tag="sc", name="pj_ps")
                    for kc in range(NKC):
                        nc.tensor.matmul(
                            ps[:, 0, :],
                            cg[:, kc, :],
                            wo_sb[:, kc, et * 512:(et + 1) * 512],
                            start=(kc == 0), stop=(kc == NKC - 1))
                    nc.vector.tensor_add(
                        o_sb[:, et * 512:(et + 1) * 512], ps[:, 0, :],
                        bo_bc[:, et * 512:(et + 1) * 512])
                r0 = (b * 2 + qp) * 128
                nc.sync.dma_start(out=out[r0:r0 + 128, :], in_=o_sb[:])

            # ---- emission schedule ---------------------------------------
            steps = [
                ("qkv", 0), ("qkv", 1), ("qkv", 2), ("qkv", 3),
                ("attn", 0, 0), ("qkv", 4), ("attn", 0, 1), ("qkv", 5),
                ("attn", 0, 2), ("qkv", 6), ("attn", 0, 3), ("qkv", 7),
                ("attn", 1, 0), ("proj", 0, 0), ("attn", 1, 1),
                ("proj", 0, 1), ("attn", 1, 2), ("attn", 1, 3),
                ("proj", 1, 0), ("proj", 1, 1),
            ]
            for st in steps:
                if st[0] == "qkv":
                    emit_qkv(st[1])
                elif st[0] == "attn":
                    emit_attn(st[1], st[2])
                else:
                    emit_proj(st[1], st[2])
                # flush the previous attention group's deferred finish
                while pending and pending[0][0] != st:
                    pending.pop(0)[1]()
            while pending:
                pending.pop(0)[1]()

    nc.compile()
    return nc


_NC = None


def _get_program():
    global _NC
    if _NC is None:
        _NC = build_program()
    return _NC


def _bf(a):
    return np.ascontiguousarray(a).astype(ml_dtypes.bfloat16)


def kernel(x, Wq, bq, Wk, bk, Wv, bv, Wo, bo, _trace=False, _trace_kwargs=None):
    x = np.asarray(x, np.float32)
    Wq, Wk, Wv, Wo = (np.asarray(w, np.float32) for w in (Wq, Wk, Wv, Wo))
    bq, bk, bv, bo = (np.asarray(v, np.float32) for v in (bq, bk, bv, bo))

    xT = _bf(x.reshape(T, E).T)
    i = np.arange(128)
    tri = _bf((i[:, None] <= i[None, :]).astype(np.float32))

    in_maps = []
    for c in range(N_CORES):
        sl = slice(c * DPC, (c + 1) * DPC)
        w3 = np.concatenate([Wq[sl, :].T, Wk[sl, :].T, Wv[sl, :].T], axis=1)
        b3 = np.stack([bq[sl], bk[sl], bv[sl]], axis=1)
        in_maps.append({
            "xT": xT,
            "w3": _bf(w3),
            "woT": _bf(Wo.T),
            "b3": np.ascontiguousarray(b3, np.float32),
            "bvr": np.ascontiguousarray(bv[sl], np.float32),
            "bo": bo,
            "tri": tri,
        })

    nc = _get_program()
    res = run_bass_kernel_spmd(nc, in_maps, list(range(N_CORES)),
                               trace=_trace, **(_trace_kwargs or {}))
    # out[c] rows are [b, qp, s, 64]: row (b, qp, s, r) holds global token
    # b*2048 + (2*qp+s)*512 + c*64 + r.
    stacked = np.stack([res.results[i]["out"].reshape(B, 2, 2, 64, E)
                        for i in range(N_CORES)], axis=3)
    full = stacked.reshape(T, E)
    if _trace:
        return full.reshape(B, S, E), res
    return full.reshape(B, S, E)


# revision 18
# speedup vs baseline: 1.2171x; 1.1513x over previous
"""Multi-head attention (B=2, S=2048, H=16, D=64) on 8 Trainium2 NeuronCores.

Head-parallel tensor parallelism: core c owns heads {2c, 2c+1} (a 128-dim
slice of the model dim): column-parallel QKV projections and local causal
attention for its 2 heads, AllToAll of bf16 context vectors (one 512-token
chunk at a time), then each core runs the full-width Wo projection for its
own disjoint 64-token slices.

Key structural choices (v2, rebuilt from a traced v1):
- DMA order: packed qkv weights + biases + tri first (~0.8 MB), then x^T in
  1024-token chunks (2 KB contiguous rows per partition line), then Wo.
  The first projection matmul can start ~7 us in, instead of waiting ~50 us
  behind an 8 MB x load with 1 KB lines.
- Scores for the two heads run CONCURRENTLY as row-tiled K=64 matmuls
  (tile_position (0,0)/(64,0)) into the two banks of one [128,1024] PSUM
  tile; one paired Exp activation covers both banks.
- Attention-times-V uses V as the stationary operand ([keys, 64+ones]) so
  the PE streams exp columns and emits ctx^T [dims, queries] directly into
  a PSUM accumulator over key blocks -- no per-block LDWEIGHTS bound and no
  128x128 context transposes afterwards. The ones column gives the softmax
  denominator on PSUM partition 64.
- Normalization multiplies ctx^T by a DMA-broadcast reciprocal row
  (reciprocal_approx_fast), pipelined one query-group behind attention.
- V is transposed into [keys, dims] tiles by the DMA transpose XBAR
  (SBUF->SBUF), costing no PE/DVE time.
- Emission interleaves batch-0 attention with batch-1 QKV tiles and batch-1
  attention with batch-0 output projections so the scalar-engine exp stream
  hides under projection matmuls; per-512-token AllToAlls keep the final
  exchange small.
"""

import sys

sys.path.insert(0, "/opt/trn_rl_repo")

import ml_dtypes
import numpy as np

import concourse.bass as bass
import concourse.tile as tile
from concourse import bacc, mybir
from concourse.bass_utils import run_bass_kernel_spmd

N_CORES = 8
B, S, H, D = 2, 2048, 16, 64
E = H * D            # 1024
T = B * S            # 4096 tokens
DPC = 128            # dims (2 heads) per core
NKC = E // 128       # 8 contraction chunks for the projections
NTT = T // 512       # 8 token tiles of 512
NTB = T // 128       # 32 token blocks of 128
SB = S // 128        # 16 key blocks per batch

F32 = mybir.dt.float32
BF16 = mybir.dt.bfloat16
AFT = mybir.ActivationFunctionType


def build_program():
    nc = bacc.Bacc("TRN2", target_bir_lowering=False, debug=False,
                   num_devices=N_CORES)

    xT = nc.dram_tensor("xT", [E, T], BF16, kind="ExternalInput").ap()
    w3 = nc.dram_tensor("w3", [E, 3 * DPC], BF16, kind="ExternalInput").ap()
    woT = nc.dram_tensor("woT", [E, E], BF16, kind="ExternalInput").ap()
    b3 = nc.dram_tensor("b3", [DPC, 3], F32, kind="ExternalInput").ap()
    bvr = nc.dram_tensor("bvr", [DPC], F32, kind="ExternalInput").ap()
    bo = nc.dram_tensor("bo", [E], F32, kind="ExternalInput").ap()
    # 128x128 lower-triangular (k_local <= q_local) mask
    tri = nc.dram_tensor("tri", [128, 128], BF16, kind="ExternalInput").ap()
    out = nc.dram_tensor("out", [T // N_CORES, E], F32, kind="ExternalOutput").ap()

    with tile.TileContext(nc) as tc:
        with (
            tc.tile_pool(name="consts", bufs=1) as consts,
            tc.tile_pool(name="state", bufs=1) as state,
            tc.tile_pool(name="ep", bufs=3) as ep,
            tc.tile_pool(name="rp", bufs=2) as rp,
            tc.tile_pool(name="cgp", bufs=2) as cgp,
            tc.tile_pool(name="op", bufs=2) as op,
            tc.tile_pool(name="ps_s", bufs=2, space="PSUM") as ps_s,
            tc.tile_pool(name="ps_c", bufs=4, space="PSUM") as ps_c,
            tc.tile_pool(name="dram", bufs=1, space="DRAM") as dram,
        ):
            # ---- small constants first: nothing blocks on the big x load --
            w3_sb = consts.tile([128, NKC, 3 * DPC], BF16)
            nc.sync.dma_start(
                out=w3_sb[:],
                in_=bass.AP(tensor=w3.tensor, offset=w3.offset,
                            ap=[[3 * DPC, 128], [3 * DPC * 128, NKC],
                                [1, 3 * DPC]]),
            )
            b3_sb = consts.tile([128, 3], F32)
            nc.sync.dma_start(out=b3_sb[:], in_=b3[:])
            tri_sb = consts.tile([128, 128], BF16)
            nc.sync.dma_start(out=tri_sb[:], in_=tri[:])

            # exp table warm-up (ACT set load ~2.7us) off the critical path
            warm = consts.tile([128, 1], BF16)
            nc.scalar.activation(warm[:], b3_sb[:, 0:1], AFT.Exp)

            # bv broadcast MUST be queued before the big x load: the first
            # vN write waits on it
            bv_bc = consts.tile([128, DPC], F32)
            nc.sync.dma_start(
                out=bv_bc[:],
                in_=bass.AP(tensor=bvr.tensor, offset=bvr.offset,
                            ap=[[0, 128], [1, DPC]]),
            )

            # ---- x^T in 1024-token chunks: 2KB contiguous lines ----------
            x_sb = state.tile([128, NKC, T], BF16)  # full x^T in SBUF
            for ch in range(4):
                cs = slice(ch * 1024, (ch + 1) * 1024)
                for kc in range(NKC):
                    nc.sync.dma_start(
                        out=x_sb[:, kc, cs],
                        in_=xT[kc * 128:(kc + 1) * 128, cs])

            # ---- wo + bo after x (needed only at projection time) --------
            wo_sb = consts.tile([128, NKC, E], BF16)
            for kc in range(NKC):
                nc.sync.dma_start(out=wo_sb[:, kc, :],
                                  in_=woT[kc * 128:(kc + 1) * 128, :])
            bo_bc = consts.tile([128, E], F32)
            nc.sync.dma_start(
                out=bo_bc[:],
                in_=bass.AP(tensor=bo.tensor, offset=bo.offset,
                            ap=[[0, 128], [1, E]]),
            )

            # ---- persistent activations ----------------------------------
            qT_sb = state.tile([128, T], BF16)   # [2-head dims, tokens]
            kT_sb = state.tile([128, T], BF16)
            # [keys, 132]: [v_h0 64 | ones | pad | v_h1 64 | ones | pad]
            vN_sb = state.tile([128, NTB, 132], BF16)
            ctxT_sb = state.tile([128, T], BF16)  # normalized ctx^T

            nc.vector.memset(vN_sb[:, :, 64:65], 1.0)
            nc.vector.memset(vN_sb[:, :, 130:131], 1.0)

            # ---- stage A: QKV projections --------------------------------
            def emit_qkv(tt):
                ts = slice(tt * 512, (tt + 1) * 512)
                for wi, dst in ((0, qT_sb), (1, kT_sb)):
                    ps = ps_c.tile([128, 512], F32, tag="ctx", name="qkv_ps")
                    for kc in range(NKC):
                        nc.tensor.matmul(
                            ps[:], w3_sb[:, kc, wi * 128:(wi + 1) * 128],
                            x_sb[:, kc, ts],
                            start=(kc == 0), stop=(kc == NKC - 1))
                    nc.vector.tensor_scalar_add(dst[:, ts], ps[:],
                                                b3_sb[:, wi:wi + 1])
                # v directly transposed: x-block stationary -> [tokens, dims]
                for tb in range(tt * 4, tt * 4 + 4):
                    pv = ps_c.tile([128, 512], F32, tag="ctx", name="v_ps")
                    for kc in range(NKC):
                        nc.tensor.matmul(
                            pv[:, 0:DPC],
                            x_sb[:, kc, tb * 128:(tb + 1) * 128],
                            w3_sb[:, kc, 2 * 128:3 * 128],
                            start=(kc == 0), stop=(kc == NKC - 1))
                    for h in range(2):
                        nc.vector.tensor_add(
                            vN_sb[:, tb, h * 66:h * 66 + 64],
                            pv[:, h * 64:(h + 1) * 64],
                            bv_bc[:, h * 64:(h + 1) * 64])

            # ---- attention for one 512-query group -----------------------
            a2a_recv = {}
            pending = []  # (tag, fin-closure)

            def emit_attn(b, qt):
                t0 = b * S
                q0 = t0 + qt * 512
                nkb = 4 * qt + 4
                cn = [ps_c.tile([128, 512], F32, tag="ctx", name=f"cn{h}")
                      for h in range(2)]

                def emit_score_pair(kb):
                    c0 = max(kb - 4 * qt, 0) * 128
                    sp = ps_s.tile([128, 2, 512], F32, tag="sc", name="sp")
                    for h in range(2):
                        nc.tensor.matmul(
                            sp[:, h, c0:512],
                            kT_sb[h * 64:(h + 1) * 64,
                                  t0 + kb * 128:t0 + (kb + 1) * 128],
                            qT_sb[h * 64:(h + 1) * 64, q0 + c0:q0 + 512],
                            start=True, stop=True)
                    return sp

                s_tiles = {0: emit_score_pair(0)}
                for kb in range(nkb):
                    c0 = max(kb - 4 * qt, 0) * 128
                    if kb + 1 < nkb:
                        s_tiles[kb + 1] = emit_score_pair(kb + 1)
                    sp = s_tiles.pop(kb)
                    e2 = ep.tile([128, 2, 512], BF16, tag="e2", name="e2")
                    # one Exp over both heads' banks
                    nc.scalar.activation(e2[:, :, c0:512], sp[:, :, c0:512],
                                         AFT.Exp, scale=0.125)
                    if kb >= 4 * qt:  # triangular block on the diagonal
                        for h in range(2):
                            nc.vector.tensor_mul(
                                e2[:, h, c0:c0 + 128],
                                e2[:, h, c0:c0 + 128], tri_sb[:])
                    tb = b * SB + kb
                    for h in range(2):
                        nc.tensor.matmul(
                            cn[h][0:65, c0:512],
                            vN_sb[:, tb, h * 66:h * 66 + 65],
                            e2[:, h, c0:512],
                            start=(kb == 0), stop=(kb == nkb - 1),
                            skip_group_check=True)

                # evacuate PSUM immediately (releases the cn banks without
                # waiting on the broadcast round-trip), then reciprocals +
                # DRAM-bounce broadcast; the ctx multiply is deferred one
                # group (fin) so the broadcast DMA latency is hidden
                cnc = [rp.tile([65, 512], F32, tag=f"cnc{h}", name=f"cnc{h}")
                       for h in range(2)]
                rec2 = rp.tile([65, 512], F32, tag="rec2", name="rec2")
                for h in range(2):
                    nc.vector.tensor_copy(cnc[h][:], cn[h][0:65, 0:512])
                    nc.vector.reciprocal(rec2[h * 64:h * 64 + 1, :],
                                         cnc[h][64:65, :])
                recd = dram.tile([2, 512], F32, tag="recd", name="recd",
                                 bufs=4)
                for h in range(2):
                    nc.sync.dma_start(out=recd[h:h + 1, :],
                                      in_=rec2[h * 64:h * 64 + 1, :])
                rbc = [rp.tile([64, 512], F32, tag=f"rbc{h}", name=f"rbc{h}")
                       for h in range(2)]
                for h in range(2):
                    src = recd[h:h + 1, :]
                    nc.sync.dma_start(
                        out=rbc[h][:],
                        in_=bass.AP(tensor=src.tensor, offset=src.offset,
                                    ap=[[0, 64], [1, 512]]))

                def fin():
                    for h in range(2):
                        nc.vector.tensor_mul(
                            ctxT_sb[h * 64:(h + 1) * 64, q0:q0 + 512],
                            cnc[h][0:64, :],
                            rbc[h][:])
                    if qt % 2 == 1:  # half-batch done: AllToAll it
                        hf = qt // 2
                        base = b * S + hf * 1024
                        ctxd = dram.tile([N_CORES, 128, 128], BF16,
                                         tag="ctxd", name="ctxd", bufs=2)
                        for j in range(N_CORES):
                            nc.sync.dma_start(
                                out=ctxd[j],
                                in_=ctxT_sb[:, base + j * 128:
                                            base + (j + 1) * 128])
                        recv = dram.tile([N_CORES, 128, 128], BF16,
                                         tag="recv", name="recv", bufs=2)
                        nc.gpsimd.collective_compute(
                            "AllToAll",
                            mybir.AluOpType.bypass,
                            replica_groups=[list(range(N_CORES))],
                            ins=[ctxd.opt()],
                            outs=[recv.opt()],
                        )
                        a2a_recv[(b, hf)] = recv

                pending.append((("attn", b, qt), fin))

            # ---- full-width output projection for a 128-token slice ------
            def emit_proj(b, qp):
                cg = cgp.tile([128, NKC, 128], BF16, tag="cg", name="cg")
                recv = a2a_recv.pop((b, qp))
                for j in range(N_CORES):
                    nc.sync.dma_start(out=cg[:, j, :], in_=recv[j])
                o_sb = op.tile([128, E], F32, tag="o", name="o_sb")
                for et in range(2):
                    ps = ps_s.tile([128, 2, 512], F32, tag="sc", name="pj_ps")
                    for kc in range(NKC):
                        nc.tensor.matmul(
                            ps[:, 0, :],
                            cg[:, kc, :],
                            wo_sb[:, kc, et * 512:(et + 1) * 512],
                            start=(kc == 0), stop=(kc == NKC - 1))
                    nc.vector.tensor_add(
                        o_sb[:, et * 512:(et + 1) * 512], ps[:, 0, :],
                        bo_bc[:, et * 512:(et + 1) * 512])
                r0 = (b * 2 + qp) * 128
                nc.sync.dma_start(out=out[r0:r0 + 128, :], in_=o_sb[:])

            # ---- emission schedule ---------------------------------------
            steps = [
                ("qkv", 0), ("qkv", 1), ("qkv", 2), ("qkv", 3),
                ("attn", 0, 0), ("qkv", 4), ("attn", 0, 1), ("qkv", 5),
                ("attn", 0, 2), ("qkv", 6), ("attn", 0, 3), ("qkv", 7),
                ("attn", 1, 0), ("proj", 0, 0), ("attn", 1, 1),
                ("proj", 0, 1), ("attn", 1, 2), ("attn", 1, 3),
                ("proj", 1, 0), ("proj", 1, 1),
            ]
            for st in steps:
                if st[0] == "qkv":
                    emit_qkv(st[1])
                elif st[0] == "attn":
                    emit_attn(st[1], st[2])
                else:
                    emit_proj(st[1], st[2])
                # flush the previous attention group's deferred finish;
                # after the last attention group, flush everything so the
                # final AllToAll fires before the second-to-last projection
                if st == ("attn", 1, 3):
                    while pending:
                        pending.pop(0)[1]()
                while pending and pending[0][0] != st:
                    pending.pop(0)[1]()

    nc.compile()
    return nc


_NC = None


def _get_program():
    global _NC
    if _NC is None:
        _NC = build_program()
    return _NC


def _bf(a):
    return np.ascontiguousarray(a).astype(ml_dtypes.bfloat16)


def kernel(x, Wq, bq, Wk, bk, Wv, bv, Wo, bo, _trace=False, _trace_kwargs=None):
    x = np.asarray(x, np.float32)
    Wq, Wk, Wv, Wo = (np.asarray(w, np.float32) for w in (Wq, Wk, Wv, Wo))
    bq, bk, bv, bo = (np.asarray(v, np.float32) for v in (bq, bk, bv, bo))

    xT = _bf(x.reshape(T, E).T)
    i = np.arange(128)
    tri = _bf((i[:, None] <= i[None, :]).astype(np.float32))

    in_maps = []
    for c in range(N_CORES):
        sl = slice(c * DPC, (c + 1) * DPC)
        w3 = np.concatenate([Wq[sl, :].T, Wk[sl, :].T, Wv[sl, :].T], axis=1)
        b3 = np.stack([bq[sl], bk[sl], bv[sl]], axis=1)
        in_maps.append({
            "xT": xT,
            "w3": _bf(w3),
            "woT": _bf(Wo.T),
            "b3": np.ascontiguousarray(b3, np.float32),
            "bvr": np.ascontiguousarray(bv[sl], np.float32),
            "bo": bo,
            "tri": tri,
        })

    nc = _get_program()
    res = run_bass_kernel_spmd(nc, in_maps, list(range(N_CORES)),
                               trace=_trace, **(_trace_kwargs or {}))
    # out[c] rows are [b, hf, 128]: row (b, hf, r) holds global token
    # b*2048 + hf*1024 + c*128 + r.
    stacked = np.stack([res.results[i]["out"].reshape(B, 2, 128, E)
                        for i in range(N_CORES)], axis=2)
    full = stacked.reshape(T, E)
    if _trace:
        return full.reshape(B, S, E), res
    return full.reshape(B, S, E)
